# revision 2
# baseline (speedup 1.0000x reference)
"""AttnBlock (VAE-style single-head spatial attention) on 8 Trainium2 cores.

Problem: x[B=4, C=512, H=64, W=64]; qkv 1x1-conv -> attention over N=H*W=4096
tokens -> proj 1x1-conv -> residual add.

Sharding: 8 cores = 4 batch images x 2 query-halves. Each core handles the
full 4096-token context (K/V) of one image and 2048 of its queries. Per-core
x columns are rotated so the query half is always columns [0, 2048) -- the
kj context order is irrelevant (summed over), so the SPMD program is
identical on every core.

Host-side folding (all cheap 512x512 ops):
 - K-bias adds a per-query constant to every logit -> cancels in softmax.
 - V-bias contributes exactly bv to every output column (softmax rows sum to
   1) -> folded into an effective proj bias beff = proj_b + proj_w @ bv.
 - S^T[kj,qi] = x^T (Wk^T (Wq x_q + bq)) = x^T (W2 x_q + b2) with
   W2 = Wk^T Wq, b2 = Wk^T bq. Scores are computed TRANSPOSED directly from
   x -- no K tensor and no on-chip transposes; softmax reduction over kj
   becomes a ones-row matmul fused into the PV accumulation loop.
 - Logits are tiny here (|s| < ~1.5), so softmax needs no max-subtraction.

On-chip pipeline per query tile (512 queries), context loop of 32 chunks of
128 tokens: S^T chunk (4 bf16 matmuls, fp32 PSUM) -> exp on ACT (bf16 out)
-> PV accumulate (4 bf16 matmuls) + denominator accumulate (1 matmul).
Then: reciprocal, partition-broadcast, normalize, proj matmuls, fused
(+beff +x fp32 residual) DVE op, DMA out. Matmul inputs are bf16 (PE full
rate); the residual uses an exact fp32 copy of the query slice.
"""

import os

import numpy as np

B, C = 4, 512
N = 4096          # H*W tokens
QH = N // 2       # queries per core
QT = 512          # query tile (free dim of most matmuls)
NQT = QH // QT    # 4 query tiles per core
NKC = N // 128    # 32 context chunks
NCC = C // 128    # 4 channel chunks
NCORES = 8

_COMPILED = None
LAST_RESULTS = None  # stashed BassKernelResults for test harness inspection


def _build():
    import concourse.bass as bass  # noqa: F401
    import concourse.mybir as mybir
    import concourse.tile as tile
    from concourse import bacc

    f32 = mybir.dt.float32
    bf16 = mybir.dt.bfloat16
    ADD = mybir.AluOpType.add
    EXP = mybir.ActivationFunctionType.Exp
    scale = float(C) ** -0.5

    nc = bacc.Bacc("TRN2", target_bir_lowering=False, debug=False,
                   num_devices=NCORES)

    # DRAM I/O (per-core shapes)
    xin = nc.dram_tensor("xin", [C, N], bf16, kind="ExternalInput")
    xq32 = nc.dram_tensor("xq32", [C, QH], f32, kind="ExternalInput")
    w2T = nc.dram_tensor("w2T", [C, C], bf16, kind="ExternalInput")
    wvT = nc.dram_tensor("wvT", [C, C], bf16, kind="ExternalInput")
    pwT = nc.dram_tensor("pwT", [C, C], bf16, kind="ExternalInput")
    b2 = nc.dram_tensor("b2", [C], f32, kind="ExternalInput")
    beff = nc.dram_tensor("beff", [C], f32, kind="ExternalInput")
    y = nc.dram_tensor("y", [C, QH], f32, kind="ExternalOutput")

    xr = xin.ap().rearrange("(t p) n -> p t n", p=128)      # [128, 4, 4096]
    xqr = xq32.ap().rearrange("(t p) n -> p t n", p=128)    # [128, 4, 2048]
    yr = y.ap().rearrange("(t p) n -> p t n", p=128)        # [128, 4, 2048]

    with tile.TileContext(nc) as tc:
        with (
            tc.tile_pool(name="singles", bufs=1) as singles,
            tc.tile_pool(name="qp", bufs=2) as qp_pool,
            tc.tile_pool(name="pt", bufs=4) as pt_pool,
            tc.tile_pool(name="hms", bufs=2) as hms_pool,
            tc.tile_pool(name="xres", bufs=2) as xres_pool,
            tc.tile_pool(name="outp", bufs=2) as out_pool,
            tc.tile_pool(name="rc", bufs=2) as rc_pool,
            tc.tile_pool(name="work", bufs=3, space="PSUM") as work_pool,
            tc.tile_pool(name="hm", bufs=1, space="PSUM") as hm_pool,
            tc.tile_pool(name="den", bufs=1, space="PSUM") as den_pool,
        ):
            # --- constant / weight loads -------------------------------
            w2T_sb = singles.tile([128, NCC, C], bf16)
            nc.sync.dma_start(out=w2T_sb,
                              in_=w2T.ap().rearrange("(t p) m -> p t m", p=128))
            wvT_sb = singles.tile([128, NCC, C], bf16)
            nc.sync.dma_start(out=wvT_sb,
                              in_=wvT.ap().rearrange("(t p) m -> p t m", p=128))
            pwT_sb = singles.tile([128, NCC, C], bf16)
            nc.sync.dma_start(out=pwT_sb,
                              in_=pwT.ap().rearrange("(t p) m -> p t m", p=128))
            b2_sb = singles.tile([128, NCC], f32)
            nc.sync.dma_start(out=b2_sb,
                              in_=b2.ap().rearrange("(t p) -> p t", p=128))
            beff_sb = singles.tile([128, NCC], f32)
            nc.sync.dma_start(out=beff_sb,
                              in_=beff.ap().rearrange("(t p) -> p t", p=128))
            ones_sb = singles.tile([128, 1], bf16)
            nc.vector.memset(ones_sb, 1.0)

            # x (bf16): 4 channel-chunks x (query half | context half)
            xa = []  # columns 0:2048 (query half = kj chunks 0..15)
            xb = []  # columns 2048:4096 (kj chunks 16..31)
            for t in range(NCC):
                xat = singles.tile([128, QH], bf16, name=f"xa{t}")
                nc.sync.dma_start(out=xat, in_=xr[:, t, 0:QH])
                xa.append(xat)
            for t in range(NCC):
                xbt = singles.tile([128, QH], bf16, name=f"xb{t}")
                nc.sync.dma_start(out=xbt, in_=xr[:, t, QH:N])
                xb.append(xbt)

            def xchunk(j):  # lhsT [ci-part, kj-cols] for context chunk j
                if j < 16:
                    return xa, j * 128
                return xb, (j - 16) * 128

            # --- V^T precompute: VT[kj, co] ----------------------------
            vt_sb = singles.tile([128, NKC, C], bf16)
            for j in range(NKC):
                xs, off = xchunk(j)
                vt_ps = work_pool.tile([128, C], f32, tag="work", name="vt_ps")
                for t in range(NCC):
                    nc.tensor.matmul(
                        vt_ps,
                        lhsT=xs[t][:, off:off + 128],
                        rhs=wvT_sb[:, t, :],
                        start=(t == 0), stop=(t == NCC - 1),
                    )
                nc.vector.tensor_copy(vt_sb[:, j, :], vt_ps)

            # --- attention over query tiles ----------------------------
            for q in range(NQT):
                qs = q * QT

                # Q' = W2 @ x_q + b2   [ci, qi-tile]  (bf16)
                qp_sb = qp_pool.tile([128, NCC, QT], bf16, tag="qp")
                for m in range(NCC):
                    qp_ps = work_pool.tile([128, QT], f32, tag="work",
                                           name="qp_ps")
                    for t in range(NCC):
                        nc.tensor.matmul(
                            qp_ps,
                            lhsT=w2T_sb[:, t, m * 128:(m + 1) * 128],
                            rhs=xa[t][:, qs:qs + QT],
                            start=(t == 0), stop=(t == NCC - 1),
                        )
                    nc.vector.tensor_scalar_add(
                        qp_sb[:, m, :], qp_ps, b2_sb[:, m:m + 1])

                hm_ps = hm_pool.tile([128, NCC, QT], f32, tag="hm")
                den_ps = den_pool.tile([1, QT], f32, tag="den")

                for j in range(NKC):
                    xs, off = xchunk(j)
                    st_ps = work_pool.tile([128, QT], f32, tag="work",
                                           name="st_ps")
                    for t in range(NCC):
                        nc.tensor.matmul(
                            st_ps,
                            lhsT=xs[t][:, off:off + 128],
                            rhs=qp_sb[:, t, :],
                            start=(t == 0), stop=(t == NCC - 1),
                        )
                    pt_sb = pt_pool.tile([128, QT], bf16, tag="pt",
                                         name="pt_sb")
                    nc.scalar.activation(pt_sb, st_ps, EXP, scale=scale)
                    for m in range(NCC):
                        nc.tensor.matmul(
                            hm_ps[:, m, :],
                            lhsT=vt_sb[:, j, m * 128:(m + 1) * 128],
                            rhs=pt_sb,
                            start=(j == 0), stop=(j == NKC - 1),
                            skip_group_check=True,
                        )
                    nc.tensor.matmul(
                        den_ps,
                        lhsT=ones_sb,
                        rhs=pt_sb,
                        start=(j == 0), stop=(j == NKC - 1),
                        skip_group_check=True,
                    )

                # normalize + proj + residual
                rec_sb = rc_pool.tile([1, QT], f32, tag="rec")
                nc.vector.reciprocal(rec_sb, den_ps)
                rbc_sb = rc_pool.tile([128, QT], f32, tag="rbc")
                nc.gpsimd.partition_broadcast(rbc_sb, rec_sb)

                hmat_sb = hms_pool.tile([128, NCC, QT], bf16, tag="hms")
                for m in range(NCC):
                    nc.vector.tensor_mul(hmat_sb[:, m, :], hm_ps[:, m, :],
                                         rbc_sb)

                xres_sb = xres_pool.tile([128, NCC, QT], f32, tag="xres")
                nc.sync.dma_start(out=xres_sb, in_=xqr[:, :, qs:qs + QT])

                out_sb = out_pool.tile([128, NCC, QT], f32, tag="out")
                for o in range(NCC):
                    pr_ps = work_pool.tile([128, QT], f32, tag="work",
                                           name="pr_ps")
                    for t in range(NCC):
                        nc.tensor.matmul(
                            pr_ps,
                            lhsT=pwT_sb[:, t, o * 128:(o + 1) * 128],
                            rhs=hmat_sb[:, t, :],
                            start=(t == 0), stop=(t == NCC - 1),
                        )
                    nc.vector.scalar_tensor_tensor(
                        out=out_sb[:, o, :],
                        in0=pr_ps,
                        scalar=beff_sb[:, o:o + 1],
                        in1=xres_sb[:, o, :],
                        op0=ADD, op1=ADD,
                    )
                nc.sync.dma_start(out=yr[:, :, qs:qs + QT], in_=out_sb)

    nc.compile()
    return nc


def _get_compiled():
    global _COMPILED
    if _COMPILED is None:
        _COMPILED = _build()
    return _COMPILED


def kernel(x, qkv_w, qkv_b, proj_w, proj_b):
    global LAST_RESULTS
    import ml_dtypes
    from concourse.bass_utils import run_bass_kernel_spmd

    bf = ml_dtypes.bfloat16
    x = np.asarray(x, dtype=np.float32)
    qkv_w = np.asarray(qkv_w, dtype=np.float32)
    qkv_b = np.asarray(qkv_b, dtype=np.float32)
    proj_w = np.asarray(proj_w, dtype=np.float32)
    proj_b = np.asarray(proj_b, dtype=np.float32)

    wq, wk, wv = qkv_w[:C], qkv_w[C:2 * C], qkv_w[2 * C:]
    bq, bv = qkv_b[:C], qkv_b[2 * C:]

    # Host-folded operands (see module docstring).
    w2T = np.ascontiguousarray((wq.T @ wk).astype(bf))   # (Wk^T Wq)^T
    b2 = np.ascontiguousarray(wk.T @ bq)
    wvT = np.ascontiguousarray(wv.T.astype(bf))
    pwT = np.ascontiguousarray(proj_w.T.astype(bf))
    beff = np.ascontiguousarray(proj_b + proj_w @ bv)

    nc = _get_compiled()

    in_maps = []
    for core in range(NCORES):
        b, h = core // 2, core % 2
        xf = x[b].reshape(C, N)
        xq = np.ascontiguousarray(xf[:, h * QH:(h + 1) * QH])
        if h == 0:
            xperm = xf.astype(bf)
        else:
            xperm = np.concatenate([xf[:, QH:], xf[:, :QH]],
                                   axis=1).astype(bf)
        in_maps.append({
            "xin": np.ascontiguousarray(xperm), "xq32": xq,
            "w2T": w2T, "wvT": wvT, "pwT": pwT, "b2": b2, "beff": beff,
        })

    res = run_bass_kernel_spmd(
        nc, in_maps, core_ids=list(range(NCORES)),
        trace=bool(os.environ.get("BASS_KERNEL_TRACE")),
    )
    LAST_RESULTS = res

    out = np.empty((B, C, N), dtype=np.float32)
    for core in range(NCORES):
        b, h = core // 2, core % 2
        out[b, :, h * QH:(h + 1) * QH] = res.results[core]["y"]
    return out.reshape(B, C, 64, 64)


# revision 6
# speedup vs baseline: 1.0093x; 1.0093x over previous
"""AttnBlock (VAE-style single-head spatial attention) on 8 Trainium2 cores.

Problem: x[B=4, C=512, H=64, W=64]; qkv 1x1-conv -> attention over N=H*W=4096
tokens -> proj 1x1-conv -> residual add.

Sharding: 8 cores = 4 batch images x 2 query-halves. Each core handles the
full 4096-token context (K/V) of one image and 2048 of its queries. Per-core
x columns are rotated so the query half is always columns [0, 2048) -- the
kj context order is irrelevant (summed over), so the SPMD program is
identical on every core.

Host-side folding (all cheap 512x512 ops):
 - K-bias adds a per-query constant to every logit -> cancels in softmax.
 - V-bias contributes exactly bv to every output column (softmax rows sum to
   1) -> folded into an effective proj bias beff = proj_b + proj_w @ bv.
 - S^T[kj,qi] = x^T (Wk^T (Wq x_q + bq)) = x^T (W2 x_q + b2) with
   W2 = Wk^T Wq, b2 = Wk^T bq. Scores are computed TRANSPOSED directly from
   x -- no K tensor and no on-chip transposes; softmax reduction over kj
   becomes a ones-row matmul fused into the PV accumulation loop.
 - Logits are tiny here (|s| < ~1.5), so softmax needs no max-subtraction.

On-chip pipeline per query tile (512 queries), context loop of 32 chunks of
128 tokens: S^T chunk (4 bf16 matmuls, fp32 PSUM) -> exp on ACT (bf16 out)
-> PV accumulate (4 bf16 matmuls) + denominator accumulate (1 matmul).
Normalization: reciprocal on DVE, partition-broadcast via a K=1 matmul,
normalize on DVE, proj matmuls, fused (+beff +x fp32 residual) DVE op.
Query tiles are software-pipelined (Q'/S^T of tile q+1 overlap the
normalize/proj of tile q) so the PE never waits on the softmax epilogue.
Matmul inputs are bf16 (PE full rate); the residual adds an exact fp32 copy
of the query slice, so output precision is dominated by fp32 rounding.
"""

import os

import numpy as np

B, C = 4, 512
N = 4096          # H*W tokens
QH = N // 2       # queries per core
QT = 512          # query tile (free dim of most matmuls)
NQT = QH // QT    # 4 query tiles per core
NKC = N // 128    # 32 context chunks
NCC = C // 128    # 4 channel chunks
NCORES = 8

_COMPILED = None
LAST_RESULTS = None  # stashed BassKernelResults for test harness inspection


def _build():
    import concourse.bass as bass  # noqa: F401
    import concourse.mybir as mybir
    import concourse.tile as tile
    from concourse import bacc

    f32 = mybir.dt.float32
    bf16 = mybir.dt.bfloat16
    ADD = mybir.AluOpType.add
    EXP = mybir.ActivationFunctionType.Exp
    scale = float(C) ** -0.5

    nc = bacc.Bacc("TRN2", target_bir_lowering=False, debug=False,
                   num_devices=NCORES)

    # DRAM I/O (per-core shapes)
    xin = nc.dram_tensor("xin", [C, N], bf16, kind="ExternalInput")
    xq32 = nc.dram_tensor("xq32", [C, QH], f32, kind="ExternalInput")
    w2T = nc.dram_tensor("w2T", [C, C], bf16, kind="ExternalInput")
    wvT = nc.dram_tensor("wvT", [C, C], bf16, kind="ExternalInput")
    pwT = nc.dram_tensor("pwT", [C, C], bf16, kind="ExternalInput")
    b2 = nc.dram_tensor("b2", [C], f32, kind="ExternalInput")
    beff = nc.dram_tensor("beff", [C], f32, kind="ExternalInput")
    y = nc.dram_tensor("y", [C, QH], f32, kind="ExternalOutput")

    xr = xin.ap().rearrange("(t p) n -> p t n", p=128)      # [128, 4, 4096]
    xqr = xq32.ap().rearrange("(t p) n -> p t n", p=128)    # [128, 4, 2048]
    yr = y.ap().rearrange("(t p) n -> p t n", p=128)        # [128, 4, 2048]

    with tile.TileContext(nc) as tc:
        with (
            tc.tile_pool(name="singles", bufs=1) as singles,
            tc.tile_pool(name="qp", bufs=2) as qp_pool,
            tc.tile_pool(name="pt", bufs=4) as pt_pool,
            tc.tile_pool(name="hms", bufs=2) as hms_pool,
            tc.tile_pool(name="xres", bufs=2) as xres_pool,
            tc.tile_pool(name="outp", bufs=2) as out_pool,
            tc.tile_pool(name="rc", bufs=2) as rc_pool,
            tc.tile_pool(name="work", bufs=3, space="PSUM") as work_pool,
            tc.tile_pool(name="hm", bufs=1, space="PSUM") as hm_pool,
            tc.tile_pool(name="den", bufs=1, space="PSUM") as den_pool,
        ):
            # --- weights / constants (DMA order = priority order) ------
            w2T_sb = singles.tile([128, NCC, C], bf16)
            nc.sync.dma_start(out=w2T_sb,
                              in_=w2T.ap().rearrange("(t p) m -> p t m", p=128))
            b2_sb = singles.tile([128, NCC], f32)
            nc.sync.dma_start(out=b2_sb,
                              in_=b2.ap().rearrange("(t p) -> p t", p=128))

            # x (bf16): [half][ci-chunk t][512-col group g] -> [128, 512]
            xt = [[[None] * 4 for _ in range(NCC)] for _ in range(2)]
            for h in range(2):
                for g in range(4):
                    for t in range(NCC):
                        xx = singles.tile([128, QT], bf16,
                                          name=f"x{h}{t}{g}")
                        col = h * QH + g * QT
                        nc.sync.dma_start(out=xx,
                                          in_=xr[:, t, col:col + QT])
                        xt[h][t][g] = xx

            wvT_sb = singles.tile([128, NCC, C], bf16)
            nc.sync.dma_start(out=wvT_sb,
                              in_=wvT.ap().rearrange("(t p) m -> p t m", p=128))
            pwT_sb = singles.tile([128, NCC, C], bf16)
            nc.sync.dma_start(out=pwT_sb,
                              in_=pwT.ap().rearrange("(t p) m -> p t m", p=128))
            beff_sb = singles.tile([128, NCC], f32)
            nc.sync.dma_start(out=beff_sb,
                              in_=beff.ap().rearrange("(t p) -> p t", p=128))
            ones_sb = singles.tile([128, 1], bf16)
            nc.vector.memset(ones_sb, 1.0)

            def xchunk(j):  # lhsT [ci-part, kj-cols] for context chunk j
                h, r = divmod(j, 16)
                g, o = divmod(r, 4)
                return (lambda t: xt[h][t][g][:, o * 128:(o + 1) * 128])

            # --- V^T precompute: VT[kj, co] ----------------------------
            vt_sb = singles.tile([128, NKC, C], bf16)
            for j in range(NKC):
                xs = xchunk(j)
                vt_ps = work_pool.tile([128, C], f32, tag="work", name="vt_ps")
                for t in range(NCC):
                    nc.tensor.matmul(
                        vt_ps, lhsT=xs(t), rhs=wvT_sb[:, t, :],
                        start=(t == 0), stop=(t == NCC - 1),
                    )
                nc.vector.tensor_copy(vt_sb[:, j, :], vt_ps)

            # --- software-pipelined query tiles ------------------------
            S = {}  # per-q live tiles

            def emit_A(q):  # Q' = W2 @ x_q + b2 ; xres prefetch
                qp_sb = qp_pool.tile([128, NCC, QT], bf16, tag="qp",
                                     name=f"qp{q}")
                for m in range(NCC):
                    qp_ps = work_pool.tile([128, QT], f32, tag="work",
                                           name="qp_ps")
                    for t in range(NCC):
                        nc.tensor.matmul(
                            qp_ps,
                            lhsT=w2T_sb[:, t, m * 128:(m + 1) * 128],
                            rhs=xt[0][t][q],
                            start=(t == 0), stop=(t == NCC - 1),
                        )
                    nc.vector.tensor_scalar_add(
                        qp_sb[:, m, :], qp_ps, b2_sb[:, m:m + 1])
                xres_sb = xres_pool.tile([128, NCC, QT], f32, tag="xres",
                                         name=f"xres{q}")
                nc.sync.dma_start(out=xres_sb,
                                  in_=xqr[:, :, q * QT:(q + 1) * QT])
                S[q] = {"qp": qp_sb, "xres": xres_sb}

            def emit_B(q, j):  # one context chunk
                if j == 0:
                    S[q]["hm"] = hm_pool.tile([128, NCC, QT], f32, tag="hm",
                                              name=f"hm{q}")
                    S[q]["den"] = den_pool.tile([1, QT], f32, tag="den",
                                                name=f"den{q}")
                qp_sb, hm_ps, den_ps = S[q]["qp"], S[q]["hm"], S[q]["den"]
                xs = xchunk(j)
                st_ps = work_pool.tile([128, QT], f32, tag="work",
                                       name="st_ps")
                for t in range(NCC):
                    nc.tensor.matmul(
                        st_ps, lhsT=xs(t), rhs=qp_sb[:, t, :],
                        start=(t == 0), stop=(t == NCC - 1),
                    )
                pt_sb = pt_pool.tile([128, QT], bf16, tag="pt", name="pt_sb")
                nc.scalar.activation(pt_sb, st_ps, EXP, scale=scale)
                for m in range(NCC):
                    nc.tensor.matmul(
                        hm_ps[:, m, :],
                        lhsT=vt_sb[:, j, m * 128:(m + 1) * 128],
                        rhs=pt_sb,
                        start=(j == 0), stop=(j == NKC - 1),
                        skip_group_check=True,
                    )
                nc.tensor.matmul(
                    den_ps, lhsT=ones_sb, rhs=pt_sb,
                    start=(j == 0), stop=(j == NKC - 1),
                    skip_group_check=True,
                )

            def emit_C_head(q):  # reciprocal + broadcast + normalize
                rec_sb = rc_pool.tile([1, QT], f32, tag="rec",
                                      name=f"rec{q}")
                nc.vector.reciprocal(rec_sb, S[q]["den"])
                rbc_sb = rc_pool.tile([128, QT], f32, tag="rbc",
                                      name=f"rbc{q}")
                nc.gpsimd.partition_broadcast(rbc_sb, rec_sb)
                hmat_sb = hms_pool.tile([128, NCC, QT], bf16, tag="hms",
                                        name=f"hms{q}")
                for m in range(NCC):
                    nc.vector.tensor_mul(hmat_sb[:, m, :],
                                         S[q]["hm"][:, m, :], rbc_sb)
                S[q]["hmat"] = hmat_sb

            def emit_C_tail(q):  # proj + bias + residual + store
                hmat_sb, xres_sb = S[q]["hmat"], S[q]["xres"]
                out_sb = out_pool.tile([128, NCC, QT], f32, tag="out",
                                       name=f"out{q}")
                for o in range(NCC):
                    pr_ps = work_pool.tile([128, QT], f32, tag="work",
                                           name="pr_ps")
                    for t in range(NCC):
                        nc.tensor.matmul(
                            pr_ps,
                            lhsT=pwT_sb[:, t, o * 128:(o + 1) * 128],
                            rhs=hmat_sb[:, t, :],
                            start=(t == 0), stop=(t == NCC - 1),
                        )
                    nc.vector.scalar_tensor_tensor(
                        out=out_sb[:, o, :],
                        in0=pr_ps,
                        scalar=beff_sb[:, o:o + 1],
                        in1=xres_sb[:, o, :],
                        op0=ADD, op1=ADD,
                    )
                nc.sync.dma_start(out=yr[:, :, q * QT:(q + 1) * QT],
                                  in_=out_sb)
                del S[q]

            OVERLAP = 2  # B(q+1) chunks emitted between C_head(q), C_tail(q)
            emit_A(0)
            for j in range(NKC):
                emit_B(0, j)
            for q in range(NQT):
                if q + 1 < NQT:
                    emit_A(q + 1)
                emit_C_head(q)
                if q + 1 < NQT:
                    for j in range(OVERLAP):
                        emit_B(q + 1, j)
                emit_C_tail(q)
                if q + 1 < NQT:
                    for j in range(OVERLAP, NKC):
                        emit_B(q + 1, j)

    nc.compile()
    return nc


def _get_compiled():
    global _COMPILED
    if _COMPILED is None:
        _COMPILED = _build()
    return _COMPILED


def kernel(x, qkv_w, qkv_b, proj_w, proj_b):
    global LAST_RESULTS
    import ml_dtypes
    from concourse.bass_utils import run_bass_kernel_spmd

    bf = ml_dtypes.bfloat16
    x = np.asarray(x, dtype=np.float32)
    qkv_w = np.asarray(qkv_w, dtype=np.float32)
    qkv_b = np.asarray(qkv_b, dtype=np.float32)
    proj_w = np.asarray(proj_w, dtype=np.float32)
    proj_b = np.asarray(proj_b, dtype=np.float32)

    wq, wk, wv = qkv_w[:C], qkv_w[C:2 * C], qkv_w[2 * C:]
    bq, bv = qkv_b[:C], qkv_b[2 * C:]

    # Host-folded operands (see module docstring).
    w2T = np.ascontiguousarray((wq.T @ wk).astype(bf))   # (Wk^T Wq)^T
    b2 = np.ascontiguousarray(wk.T @ bq)
    wvT = np.ascontiguousarray(wv.T.astype(bf))
    pwT = np.ascontiguousarray(proj_w.T.astype(bf))
    beff = np.ascontiguousarray(proj_b + proj_w @ bv)

    nc = _get_compiled()

    in_maps = []
    for core in range(NCORES):
        b, h = core // 2, core % 2
        xf = x[b].reshape(C, N)
        xq = np.ascontiguousarray(xf[:, h * QH:(h + 1) * QH])
        if h == 0:
            xperm = xf.astype(bf)
        else:
            xperm = np.concatenate([xf[:, QH:], xf[:, :QH]],
                                   axis=1).astype(bf)
        in_maps.append({
            "xin": np.ascontiguousarray(xperm), "xq32": xq,
            "w2T": w2T, "wvT": wvT, "pwT": pwT, "b2": b2, "beff": beff,
        })

    res = run_bass_kernel_spmd(
        nc, in_maps, core_ids=list(range(NCORES)),
        trace=bool(os.environ.get("BASS_KERNEL_TRACE")),
    )
    LAST_RESULTS = res

    out = np.empty((B, C, N), dtype=np.float32)
    for core in range(NCORES):
        b, h = core // 2, core % 2
        out[b, :, h * QH:(h + 1) * QH] = res.results[core]["y"]
    return out.reshape(B, C, 64, 64)


# revision 7
# speedup vs baseline: 1.2072x; 1.1960x over previous
"""AttnBlock (VAE-style single-head spatial attention) on 8 Trainium2 cores.

Problem: x[B=4, C=512, H=64, W=64]; qkv 1x1-conv -> attention over N=H*W=4096
tokens -> proj 1x1-conv -> residual add.

Sharding: 8 cores = 4 batch images x 2 query-halves. Each core handles the
full 4096-token context (K/V) of one image and 2048 of its queries. Per-core
x columns are rotated so the query half is always columns [0, 2048) -- the
kj context order is irrelevant (summed over), so the SPMD program is
identical on every core.

Host-side folding (all cheap 512x512 ops):
 - K-bias adds a per-query constant to every logit -> cancels in softmax.
 - V-bias contributes exactly bv to every output column (softmax rows sum to
   1) -> folded into an effective proj bias beff = proj_b + proj_w @ bv.
 - S^T[kj,qi] = x^T (Wk^T (Wq x_q + bq)) = x^T (W2 x_q + b2) with
   W2 = Wk^T Wq, b2 = Wk^T bq. Scores are computed TRANSPOSED directly from
   x -- no K tensor and no on-chip transposes.
 - Logits are tiny here (|s| < ~1.5), so softmax needs no max-subtraction.

On-chip structure per query tile (512 queries), context loop of 32 chunks
of 128 tokens: S^T chunk (4 bf16 matmuls, fp32 PSUM) -> exp on ACT (bf16
out) -> PV accumulate (4 bf16 matmuls into PSUM) + DVE accumulation of
sum-exp partials. The softmax denominator finishes with a single
ones-vector matmul per tile; normalization multiplies a
partition-broadcast reciprocal; proj matmuls; a fused
(+beff +x fp32 residual) DVE op writes the output. V^T is produced
just-in-time inside the first tile's context loop, and query tiles are
software-pipelined so the PE never idles on the softmax epilogue. All
matmul inputs are bf16 (PE full rate); the residual adds an exact fp32
copy of the query slice, so output precision is residual-dominated.
"""

import os

import numpy as np

B, C = 4, 512
N = 4096          # H*W tokens
QH = N // 2       # queries per core
QT = 512          # query tile (free dim of most matmuls)
NQT = QH // QT    # 4 query tiles per core
NKC = N // 128    # 32 context chunks
NCC = C // 128    # 4 channel chunks
NCORES = 8
OVERLAP = 3       # next-tile chunks emitted inside the epilogue window

_COMPILED = None
LAST_RESULTS = None  # stashed BassKernelResults for test harness inspection


def _build():
    import concourse.bass as bass  # noqa: F401
    import concourse.mybir as mybir
    import concourse.tile as tile
    from concourse import bacc

    f32 = mybir.dt.float32
    bf16 = mybir.dt.bfloat16
    ADD = mybir.AluOpType.add
    EXP = mybir.ActivationFunctionType.Exp
    scale = float(C) ** -0.5

    nc = bacc.Bacc("TRN2", target_bir_lowering=False, debug=False,
                   num_devices=NCORES)

    # DRAM I/O (per-core shapes)
    xin = nc.dram_tensor("xin", [C, N], bf16, kind="ExternalInput")
    xq32 = nc.dram_tensor("xq32", [C, QH], f32, kind="ExternalInput")
    w2T = nc.dram_tensor("w2T", [C, C], bf16, kind="ExternalInput")
    wvT = nc.dram_tensor("wvT", [C, C], bf16, kind="ExternalInput")
    pwT = nc.dram_tensor("pwT", [C, C], bf16, kind="ExternalInput")
    b2 = nc.dram_tensor("b2", [C], f32, kind="ExternalInput")
    beff = nc.dram_tensor("beff", [C], f32, kind="ExternalInput")
    y = nc.dram_tensor("y", [C, QH], f32, kind="ExternalOutput")

    xr = xin.ap().rearrange("(t p) n -> p t n", p=128)      # [128, 4, 4096]
    xqr = xq32.ap().rearrange("(t p) n -> p t n", p=128)    # [128, 4, 2048]
    yr = y.ap().rearrange("(t p) n -> p t n", p=128)        # [128, 4, 2048]

    with tile.TileContext(nc) as tc:
        with (
            tc.tile_pool(name="singles", bufs=1) as singles,
            tc.tile_pool(name="qp", bufs=2) as qp_pool,
            tc.tile_pool(name="pt", bufs=4) as pt_pool,
            tc.tile_pool(name="hms", bufs=2) as hms_pool,
            tc.tile_pool(name="dacc", bufs=2) as dacc_pool,
            tc.tile_pool(name="xres", bufs=2) as xres_pool,
            tc.tile_pool(name="outp", bufs=2) as out_pool,
            tc.tile_pool(name="rc", bufs=2) as rc_pool,
            tc.tile_pool(name="work", bufs=4, space="PSUM") as work_pool,
            tc.tile_pool(name="hm", bufs=1, space="PSUM") as hm_pool,
        ):
            # --- DMAs in consumption-priority order ---------------------
            w2T_sb = singles.tile([128, NCC, C], bf16)
            nc.sync.dma_start(out=w2T_sb,
                              in_=w2T.ap().rearrange("(t p) m -> p t m", p=128))
            b2_sb = singles.tile([128, NCC], f32)
            nc.sync.dma_start(out=b2_sb,
                              in_=b2.ap().rearrange("(t p) -> p t", p=128))

            # x (bf16): [half][ci-chunk t][512-col group g] -> [128, 512]
            xt = [[[None] * 4 for _ in range(NCC)] for _ in range(2)]

            def load_x(h, g):
                for t in range(NCC):
                    xx = singles.tile([128, QT], bf16, name=f"x{h}{t}{g}")
                    col = h * QH + g * QT
                    nc.sync.dma_start(out=xx, in_=xr[:, t, col:col + QT])
                    xt[h][t][g] = xx

            load_x(0, 0)                      # Q'(0) + first context chunks
            wvT_sb = singles.tile([128, NCC, C], bf16)
            nc.sync.dma_start(out=wvT_sb,
                              in_=wvT.ap().rearrange("(t p) m -> p t m", p=128))
            for g in range(1, 4):
                load_x(0, g)
            for g in range(4):
                load_x(1, g)
            pwT_sb = singles.tile([128, NCC, C], bf16)
            nc.sync.dma_start(out=pwT_sb,
                              in_=pwT.ap().rearrange("(t p) m -> p t m", p=128))
            beff_sb = singles.tile([128, NCC], f32)
            nc.sync.dma_start(out=beff_sb,
                              in_=beff.ap().rearrange("(t p) -> p t", p=128))
            ones_sb = singles.tile([128, 1], f32)
            nc.vector.memset(ones_sb, 1.0)

            def xchunk(j):  # lhsT [ci-part, kj-cols] for context chunk j
                h, r = divmod(j, 16)
                g, o = divmod(r, 4)
                return (lambda t: xt[h][t][g][:, o * 128:(o + 1) * 128])

            vt_sb = singles.tile([128, NKC, C], bf16)

            S = {}  # per-q live tiles

            def emit_A(q):  # Q' = W2 @ x_q + b2
                qp_sb = qp_pool.tile([128, NCC, QT], bf16, tag="qp",
                                     name=f"qp{q}")
                for m in range(NCC):
                    qp_ps = work_pool.tile([128, QT], f32, tag="work",
                                           name="qp_ps")
                    for t in range(NCC):
                        nc.tensor.matmul(
                            qp_ps,
                            lhsT=w2T_sb[:, t, m * 128:(m + 1) * 128],
                            rhs=xt[0][t][q],
                            start=(t == 0), stop=(t == NCC - 1),
                        )
                    nc.vector.tensor_scalar_add(
                        qp_sb[:, m, :], qp_ps, b2_sb[:, m:m + 1])
                S[q] = {"qp": qp_sb}

            def emit_B(q, j):  # one context chunk
                if j == 0:
                    S[q]["hm"] = hm_pool.tile([128, NCC, QT], f32, tag="hm",
                                              name=f"hm{q}")
                    S[q]["dacc"] = dacc_pool.tile([128, QT], f32, tag="dacc",
                                                  name=f"dacc{q}")
                if j == 8:  # prefetch fp32 residual slice mid-loop
                    xres_sb = xres_pool.tile([128, NCC, QT], f32, tag="xres",
                                             name=f"xres{q}")
                    nc.sync.dma_start(
                        out=xres_sb, in_=xqr[:, :, q * QT:(q + 1) * QT])
                    S[q]["xres"] = xres_sb
                qp_sb, hm_ps = S[q]["qp"], S[q]["hm"]
                xs = xchunk(j)
                if q == 0:  # V^T produced just-in-time in tile 0's loop
                    vt_ps = work_pool.tile([128, C], f32, tag="work",
                                           name="vt_ps")
                    for t in range(NCC):
                        nc.tensor.matmul(
                            vt_ps, lhsT=xs(t), rhs=wvT_sb[:, t, :],
                            start=(t == 0), stop=(t == NCC - 1),
                        )
                    nc.vector.tensor_copy(vt_sb[:, j, :], vt_ps)
                st_ps = work_pool.tile([128, QT], f32, tag="work",
                                       name="st_ps")
                for t in range(NCC):
                    nc.tensor.matmul(
                        st_ps, lhsT=xs(t), rhs=qp_sb[:, t, :],
                        start=(t == 0), stop=(t == NCC - 1),
                    )
                pt_sb = pt_pool.tile([128, QT], bf16, tag="pt", name="pt_sb")
                nc.scalar.activation(pt_sb, st_ps, EXP, scale=scale)
                for m in range(NCC):
                    nc.tensor.matmul(
                        hm_ps[:, m, :],
                        lhsT=vt_sb[:, j, m * 128:(m + 1) * 128],
                        rhs=pt_sb,
                        start=(j == 0), stop=(j == NKC - 1),
                        skip_group_check=True,
                    )
                dacc = S[q]["dacc"]
                if j == 0:
                    nc.vector.tensor_copy(dacc, pt_sb)
                else:
                    nc.vector.tensor_add(dacc, dacc, pt_sb)

            def emit_C_head(q):  # denominator + normalize
                den_ps = work_pool.tile([1, QT], f32, tag="work",
                                        name="den_ps")
                nc.tensor.matmul(den_ps, lhsT=ones_sb, rhs=S[q]["dacc"])
                rec_sb = rc_pool.tile([1, QT], f32, tag="rec",
                                      name=f"rec{q}")
                nc.vector.reciprocal(rec_sb, den_ps)
                rbc_sb = rc_pool.tile([128, QT], f32, tag="rbc",
                                      name=f"rbc{q}")
                nc.gpsimd.partition_broadcast(rbc_sb, rec_sb)
                hmat_sb = hms_pool.tile([128, NCC, QT], bf16, tag="hms",
                                        name=f"hms{q}")
                for m in range(NCC):
                    nc.vector.tensor_mul(hmat_sb[:, m, :],
                                         S[q]["hm"][:, m, :], rbc_sb)
                S[q]["hmat"] = hmat_sb

            def emit_C_tail(q):  # proj + bias + residual + store
                hmat_sb, xres_sb = S[q]["hmat"], S[q]["xres"]
                out_sb = out_pool.tile([128, NCC, QT], f32, tag="out",
                                       name=f"out{q}")
                for o in range(NCC):
                    pr_ps = work_pool.tile([128, QT], f32, tag="work",
                                           name="pr_ps")
                    for t in range(NCC):
                        nc.tensor.matmul(
                            pr_ps,
                            lhsT=pwT_sb[:, t, o * 128:(o + 1) * 128],
                            rhs=hmat_sb[:, t, :],
                            start=(t == 0), stop=(t == NCC - 1),
                        )
                    nc.vector.scalar_tensor_tensor(
                        out=out_sb[:, o, :],
                        in0=pr_ps,
                        scalar=beff_sb[:, o:o + 1],
                        in1=xres_sb[:, o, :],
                        op0=ADD, op1=ADD,
                    )
                nc.sync.dma_start(out=yr[:, :, q * QT:(q + 1) * QT],
                                  in_=out_sb)
                del S[q]

            emit_A(0)
            for j in range(NKC):
                emit_B(0, j)
            for q in range(NQT):
                if q + 1 < NQT:
                    emit_A(q + 1)
                emit_C_head(q)
                if q + 1 < NQT:
                    for j in range(OVERLAP):
                        emit_B(q + 1, j)
                emit_C_tail(q)
                if q + 1 < NQT:
                    for j in range(OVERLAP, NKC):
                        emit_B(q + 1, j)

    nc.compile()
    return nc


def _get_compiled():
    global _COMPILED
    if _COMPILED is None:
        _COMPILED = _build()
    return _COMPILED


def kernel(x, qkv_w, qkv_b, proj_w, proj_b):
    global LAST_RESULTS
    import ml_dtypes
    from concourse.bass_utils import run_bass_kernel_spmd

    bf = ml_dtypes.bfloat16
    x = np.asarray(x, dtype=np.float32)
    qkv_w = np.asarray(qkv_w, dtype=np.float32)
    qkv_b = np.asarray(qkv_b, dtype=np.float32)
    proj_w = np.asarray(proj_w, dtype=np.float32)
    proj_b = np.asarray(proj_b, dtype=np.float32)

    wq, wk, wv = qkv_w[:C], qkv_w[C:2 * C], qkv_w[2 * C:]
    bq, bv = qkv_b[:C], qkv_b[2 * C:]

    # Host-folded operands (see module docstring).
    w2T = np.ascontiguousarray((wq.T @ wk).astype(bf))   # (Wk^T Wq)^T
    b2 = np.ascontiguousarray(wk.T @ bq)
    wvT = np.ascontiguousarray(wv.T.astype(bf))
    pwT = np.ascontiguousarray(proj_w.T.astype(bf))
    beff = np.ascontiguousarray(proj_b + proj_w @ bv)

    nc = _get_compiled()

    in_maps = []
    for core in range(NCORES):
        b, h = core // 2, core % 2
        xf = x[b].reshape(C, N)
        xq = np.ascontiguousarray(xf[:, h * QH:(h + 1) * QH])
        if h == 0:
            xperm = xf.astype(bf)
        else:
            xperm = np.concatenate([xf[:, QH:], xf[:, :QH]],
                                   axis=1).astype(bf)
        in_maps.append({
            "xin": np.ascontiguousarray(xperm), "xq32": xq,
            "w2T": w2T, "wvT": wvT, "pwT": pwT, "b2": b2, "beff": beff,
        })

    res = run_bass_kernel_spmd(
        nc, in_maps, core_ids=list(range(NCORES)),
        trace=bool(os.environ.get("BASS_KERNEL_TRACE")),
    )
    LAST_RESULTS = res

    out = np.empty((B, C, N), dtype=np.float32)
    for core in range(NCORES):
        b, h = core // 2, core % 2
        out[b, :, h * QH:(h + 1) * QH] = res.results[core]["y"]
    return out.reshape(B, C, 64, 64)


# revision 11
# speedup vs baseline: 1.2083x; 1.0010x over previous
"""AttnBlock (VAE-style single-head spatial attention) on 8 Trainium2 cores.

Problem: x[B=4, C=512, H=64, W=64]; qkv 1x1-conv -> attention over N=H*W=4096
tokens -> proj 1x1-conv -> residual add.

Sharding: 8 cores = 4 batch images x 2 query-halves. Each core handles the
full 4096-token context (K/V) of one image and 2048 of its queries. Per-core
x columns are rotated so the query half is always columns [0, 2048) -- the
kj context order is irrelevant (summed over), so the SPMD program is
identical on every core.

Host-side folding (all cheap 512x512 ops):
 - K-bias adds a per-query constant to every logit -> cancels in softmax.
 - V-bias contributes exactly bv to every output column (softmax rows sum to
   1) -> folded into an effective proj bias beff = proj_b + proj_w @ bv.
 - S^T[kj,qi] = x^T (Wk^T (Wq x_q + bq)) = x^T (W2 x_q + b2) with
   W2 = Wk^T Wq, b2 = Wk^T bq. Scores are computed TRANSPOSED directly from
   x -- no K tensor and no on-chip transposes.
 - Logits are tiny here (|s| < ~1.5), so softmax needs no max-subtraction.

On-chip structure per query tile (512 queries), context loop of 32 chunks
of 128 tokens: S^T chunk (4 bf16 matmuls, fp32 PSUM) -> exp on ACT (bf16
out) -> PV accumulate (4 bf16 matmuls into PSUM) + DVE accumulation of
sum-exp partials. The softmax denominator finishes with a single
ones-vector matmul per tile; normalization multiplies a
partition-broadcast reciprocal; proj matmuls; a fused
(+beff +x fp32 residual) DVE op writes the output. V^T is produced
just-in-time inside the first tile's context loop, and query tiles are
software-pipelined so the PE never idles on the softmax epilogue. All
matmul inputs are bf16 (PE full rate); the residual adds an exact fp32
copy of the query slice, so output precision is residual-dominated.
"""

import os

import numpy as np

B, C = 4, 512
N = 4096          # H*W tokens
QH = N // 2       # queries per core
QT = 512          # query tile (free dim of most matmuls)
NQT = QH // QT    # 4 query tiles per core
NKC = N // 128    # 32 context chunks
NCC = C // 128    # 4 channel chunks
NCORES = 8
OVERLAP = 3       # next-tile chunks emitted inside the epilogue window

_COMPILED = None
LAST_RESULTS = None  # stashed BassKernelResults for test harness inspection


def _build():
    import concourse.bass as bass  # noqa: F401
    import concourse.mybir as mybir
    import concourse.tile as tile
    from concourse import bacc

    f32 = mybir.dt.float32
    bf16 = mybir.dt.bfloat16
    ADD = mybir.AluOpType.add
    EXP = mybir.ActivationFunctionType.Exp
    scale = float(C) ** -0.5

    nc = bacc.Bacc("TRN2", target_bir_lowering=False, debug=False,
                   num_devices=NCORES)

    # DRAM I/O (per-core shapes)
    xin = nc.dram_tensor("xin", [C, N], bf16, kind="ExternalInput")
    xq32 = nc.dram_tensor("xq32", [C, QH], f32, kind="ExternalInput")
    w2T = nc.dram_tensor("w2T", [C, C], bf16, kind="ExternalInput")
    wvT = nc.dram_tensor("wvT", [C, C], bf16, kind="ExternalInput")
    pwT = nc.dram_tensor("pwT", [C, C], bf16, kind="ExternalInput")
    b2 = nc.dram_tensor("b2", [C], f32, kind="ExternalInput")
    beff = nc.dram_tensor("beff", [C], f32, kind="ExternalInput")
    y = nc.dram_tensor("y", [C, QH], f32, kind="ExternalOutput")

    xr = xin.ap().rearrange("(t p) n -> p t n", p=128)      # [128, 4, 4096]
    xqr = xq32.ap().rearrange("(t p) n -> p t n", p=128)    # [128, 4, 2048]
    yr = y.ap().rearrange("(t p) n -> p t n", p=128)        # [128, 4, 2048]

    with tile.TileContext(nc) as tc:
        with (
            tc.tile_pool(name="singles", bufs=1) as singles,
            tc.tile_pool(name="qp", bufs=2) as qp_pool,
            tc.tile_pool(name="pt", bufs=4) as pt_pool,
            tc.tile_pool(name="hms", bufs=2) as hms_pool,
            tc.tile_pool(name="dacc", bufs=2) as dacc_pool,
            tc.tile_pool(name="xres", bufs=2) as xres_pool,
            tc.tile_pool(name="outp", bufs=2) as out_pool,
            tc.tile_pool(name="rc", bufs=2) as rc_pool,
            tc.tile_pool(name="work", bufs=4, space="PSUM") as work_pool,
            tc.tile_pool(name="hm", bufs=1, space="PSUM") as hm_pool,
        ):
            # --- PE warmup: ~4.5us of dependency-free matmuls ----------
            # The HAM clock gate needs ~3.4us of sustained PE activity to
            # lift the 1.2 GHz cold throttle; these run during the input
            # DMA wait so the real matmuls start at 2.4 GHz.
            wu_sb = singles.tile([128, QT], bf16)
            nc.vector.memset(wu_sb, 0.0)
            ones_bf = singles.tile([128, 1], bf16)
            nc.vector.memset(ones_bf, 1.0)
            wu_keep = singles.tile([1, QT], f32)
            for w in range(20):
                wu_ps = work_pool.tile([1, QT], f32, tag="work", name="wu_ps")
                nc.tensor.matmul(wu_ps, lhsT=ones_bf, rhs=wu_sb)
                if w == 19:  # keep the chain live against DCE
                    nc.vector.tensor_copy(wu_keep, wu_ps)

            # --- DMAs in consumption-priority order ---------------------
            w2T_sb = singles.tile([128, NCC, C], bf16)
            nc.sync.dma_start(out=w2T_sb,
                              in_=w2T.ap().rearrange("(t p) m -> p t m", p=128))
            b2_sb = singles.tile([128, NCC], f32)
            nc.sync.dma_start(out=b2_sb,
                              in_=b2.ap().rearrange("(t p) -> p t", p=128))

            # x (bf16): [half][ci-chunk t][512-col group g] -> [128, 512]
            xt = [[[None] * 4 for _ in range(NCC)] for _ in range(2)]

            def load_x(h, g):
                for t in range(NCC):
                    xx = singles.tile([128, QT], bf16, name=f"x{h}{t}{g}")
                    col = h * QH + g * QT
                    nc.sync.dma_start(out=xx, in_=xr[:, t, col:col + QT])
                    xt[h][t][g] = xx

            load_x(0, 0)                      # Q'(0) + first context chunks
            wvT_sb = singles.tile([128, NCC, C], bf16)
            nc.sync.dma_start(out=wvT_sb,
                              in_=wvT.ap().rearrange("(t p) m -> p t m", p=128))
            for g in range(1, 4):
                load_x(0, g)
            for g in range(4):
                load_x(1, g)
            pwT_sb = singles.tile([128, NCC, C], bf16)
            nc.sync.dma_start(out=pwT_sb,
                              in_=pwT.ap().rearrange("(t p) m -> p t m", p=128))
            beff_sb = singles.tile([128, NCC], f32)
            nc.sync.dma_start(out=beff_sb,
                              in_=beff.ap().rearrange("(t p) -> p t", p=128))
            ones_sb = singles.tile([128, 1], f32)
            nc.vector.memset(ones_sb, 1.0)

            def xchunk(j):  # lhsT [ci-part, kj-cols] for context chunk j
                h, r = divmod(j, 16)
                g, o = divmod(r, 4)
                return (lambda t: xt[h][t][g][:, o * 128:(o + 1) * 128])

            vt_sb = singles.tile([128, NKC, C], bf16)

            S = {}  # per-q live tiles

            def emit_A(q):  # Q' = W2 @ x_q + b2
                qp_sb = qp_pool.tile([128, NCC, QT], bf16, tag="qp",
                                     name=f"qp{q}")
                for m in range(NCC):
                    qp_ps = work_pool.tile([128, QT], f32, tag="work",
                                           name="qp_ps")
                    for t in range(NCC):
                        nc.tensor.matmul(
                            qp_ps,
                            lhsT=w2T_sb[:, t, m * 128:(m + 1) * 128],
                            rhs=xt[0][t][q],
                            start=(t == 0), stop=(t == NCC - 1),
                        )
                    nc.vector.tensor_scalar_add(
                        qp_sb[:, m, :], qp_ps, b2_sb[:, m:m + 1])
                S[q] = {"qp": qp_sb}

            def emit_B_st(q, j):  # S^T + exp of one context chunk
                if j == 0:
                    S[q]["hm"] = hm_pool.tile([128, NCC, QT], f32, tag="hm",
                                              name=f"hm{q}")
                    S[q]["dacc"] = dacc_pool.tile([128, QT], f32, tag="dacc",
                                                  name=f"dacc{q}")
                    S[q]["pt"] = {}
                if j == 8:  # prefetch fp32 residual slice mid-loop
                    xres_sb = xres_pool.tile([128, NCC, QT], f32, tag="xres",
                                             name=f"xres{q}")
                    nc.sync.dma_start(
                        out=xres_sb, in_=xqr[:, :, q * QT:(q + 1) * QT])
                    S[q]["xres"] = xres_sb
                qp_sb = S[q]["qp"]
                xs = xchunk(j)
                if q == 0:  # V^T produced just-in-time in tile 0's loop
                    vt_ps = work_pool.tile([128, C], f32, tag="work",
                                           name="vt_ps")
                    for t in range(NCC):
                        nc.tensor.matmul(
                            vt_ps, lhsT=xs(t), rhs=wvT_sb[:, t, :],
                            start=(t == 0), stop=(t == NCC - 1),
                        )
                    nc.vector.tensor_copy(vt_sb[:, j, :], vt_ps)
                st_ps = work_pool.tile([128, QT], f32, tag="work",
                                       name="st_ps")
                for t in range(NCC):
                    nc.tensor.matmul(
                        st_ps, lhsT=xs(t), rhs=qp_sb[:, t, :],
                        start=(t == 0), stop=(t == NCC - 1),
                    )
                pt_sb = pt_pool.tile([128, QT], bf16, tag="pt", name="pt_sb")
                nc.scalar.activation(pt_sb, st_ps, EXP, scale=scale)
                S[q]["pt"][j] = pt_sb

            def emit_B_pv(q, j):  # PV accumulate + sum-exp accumulate
                hm_ps = S[q]["hm"]
                pt_sb = S[q]["pt"].pop(j)
                for m in range(NCC):
                    nc.tensor.matmul(
                        hm_ps[:, m, :],
                        lhsT=vt_sb[:, j, m * 128:(m + 1) * 128],
                        rhs=pt_sb,
                        start=(j == 0), stop=(j == NKC - 1),
                        skip_group_check=True,
                    )
                dacc = S[q]["dacc"]
                if j == 0:
                    nc.vector.tensor_copy(dacc, pt_sb)
                else:
                    nc.vector.tensor_add(dacc, dacc, pt_sb)

            def emit_B(q, j):
                emit_B_st(q, j)
                emit_B_pv(q, j)

            def emit_C_head(q):  # denominator + normalize
                den_ps = work_pool.tile([1, QT], f32, tag="work",
                                        name="den_ps")
                nc.tensor.matmul(den_ps, lhsT=ones_sb, rhs=S[q]["dacc"])
                rec_sb = rc_pool.tile([1, QT], f32, tag="rec",
                                      name=f"rec{q}")
                nc.vector.reciprocal(rec_sb, den_ps)
                rbc_sb = rc_pool.tile([128, QT], f32, tag="rbc",
                                      name=f"rbc{q}")
                nc.gpsimd.partition_broadcast(rbc_sb, rec_sb)
                hmat_sb = hms_pool.tile([128, NCC, QT], bf16, tag="hms",
                                        name=f"hms{q}")
                for m in range(NCC):
                    nc.vector.tensor_mul(hmat_sb[:, m, :],
                                         S[q]["hm"][:, m, :], rbc_sb)
                S[q]["hmat"] = hmat_sb

            def emit_C_tail(q):  # proj + bias + residual + store
                hmat_sb, xres_sb = S[q]["hmat"], S[q]["xres"]
                out_sb = out_pool.tile([128, NCC, QT], f32, tag="out",
                                       name=f"out{q}")
                for o in range(NCC):
                    pr_ps = work_pool.tile([128, QT], f32, tag="work",
                                           name="pr_ps")
                    for t in range(NCC):
                        nc.tensor.matmul(
                            pr_ps,
                            lhsT=pwT_sb[:, t, o * 128:(o + 1) * 128],
                            rhs=hmat_sb[:, t, :],
                            start=(t == 0), stop=(t == NCC - 1),
                        )
                    nc.vector.scalar_tensor_tensor(
                        out=out_sb[:, o, :],
                        in0=pr_ps,
                        scalar=beff_sb[:, o:o + 1],
                        in1=xres_sb[:, o, :],
                        op0=ADD, op1=ADD,
                    )
                nc.sync.dma_start(out=yr[:, :, q * QT:(q + 1) * QT],
                                  in_=out_sb)
                del S[q]

            # Pipeline: during tile q's epilogue (denominator -> normalize
            # -> proj), the PE stream holds only dependency-free work from
            # tile q+1 (Q' and S^T/exp of the first OVERLAP chunks); their
            # PV matmuls are deferred past proj so the in-order PE never
            # blocks on the epilogue's DVE/GPSIMD chain.
            emit_A(0)
            for j in range(NKC):
                emit_B(0, j)
            for q in range(NQT):
                if q + 1 < NQT:
                    emit_A(q + 1)
                emit_C_head(q)
                if q + 1 < NQT:
                    for j in range(OVERLAP):
                        emit_B_st(q + 1, j)
                emit_C_tail(q)
                if q + 1 < NQT:
                    for j in range(OVERLAP):
                        emit_B_pv(q + 1, j)
                    for j in range(OVERLAP, NKC):
                        emit_B(q + 1, j)

    nc.compile()
    return nc


def _get_compiled():
    global _COMPILED
    if _COMPILED is None:
        _COMPILED = _build()
    return _COMPILED


def kernel(x, qkv_w, qkv_b, proj_w, proj_b):
    global LAST_RESULTS
    import ml_dtypes
    from concourse.bass_utils import run_bass_kernel_spmd

    bf = ml_dtypes.bfloat16
    x = np.asarray(x, dtype=np.float32)
    qkv_w = np.asarray(qkv_w, dtype=np.float32)
    qkv_b = np.asarray(qkv_b, dtype=np.float32)
    proj_w = np.asarray(proj_w, dtype=np.float32)
    proj_b = np.asarray(proj_b, dtype=np.float32)

    wq, wk, wv = qkv_w[:C], qkv_w[C:2 * C], qkv_w[2 * C:]
    bq, bv = qkv_b[:C], qkv_b[2 * C:]

    # Host-folded operands (see module docstring).
    w2T = np.ascontiguousarray((wq.T @ wk).astype(bf))   # (Wk^T Wq)^T
    b2 = np.ascontiguousarray(wk.T @ bq)
    wvT = np.ascontiguousarray(wv.T.astype(bf))
    pwT = np.ascontiguousarray(proj_w.T.astype(bf))
    beff = np.ascontiguousarray(proj_b + proj_w @ bv)

    nc = _get_compiled()

    in_maps = []
    for core in range(NCORES):
        b, h = core // 2, core % 2
        xf = x[b].reshape(C, N)
        xq = np.ascontiguousarray(xf[:, h * QH:(h + 1) * QH])
        if h == 0:
            xperm = xf.astype(bf)
        else:
            xperm = np.concatenate([xf[:, QH:], xf[:, :QH]],
                                   axis=1).astype(bf)
        in_maps.append({
            "xin": np.ascontiguousarray(xperm), "xq32": xq,
            "w2T": w2T, "wvT": wvT, "pwT": pwT, "b2": b2, "beff": beff,
        })

    res = run_bass_kernel_spmd(
        nc, in_maps, core_ids=list(range(NCORES)),
        trace=bool(os.environ.get("BASS_KERNEL_TRACE")),
    )
    LAST_RESULTS = res

    out = np.empty((B, C, N), dtype=np.float32)
    for core in range(NCORES):
        b, h = core // 2, core % 2
        out[b, :, h * QH:(h + 1) * QH] = res.results[core]["y"]
    return out.reshape(B, C, 64, 64)
